# revision 45
# baseline (speedup 1.0000x reference)
"""TRN2 Bass kernel for nn_AttentionAttributionTransformerEncoderLayer.

Data-parallel over batch: 8 batch elements -> 8 NeuronCores, no collectives.
Per core: full post-LN transformer encoder layer on [S=1024, D=1024] plus the
[H=16, S, S] attention-weight output.

Matmul dtypes: float32r (full-rate, ~1.4e-4 rel err) for QKV / scores;
bf16 for exp/V tiles feeding attn@V and for out_proj; fp16 for both FFN
matmuls. PSUM accumulation is always f32.
"""
import sys

sys.path.insert(0, "/opt/trn_rl_repo")

from contextlib import ExitStack

import ml_dtypes
import numpy as np

import concourse.bass as bass
import concourse.tile as tile
import concourse.mybir as mybir
from concourse import bacc, bass_utils
from concourse.masks import make_identity

F32 = mybir.dt.float32
F32R = mybir.dt.float32r
F16 = mybir.dt.float16
BF16 = mybir.dt.bfloat16
AF = mybir.ActivationFunctionType
ALU = mybir.AluOpType

B, S, D = 8, 1024, 1024
H, DH = 16, 64
FF = 4096
P = 128
NT = S // P       # 8 l/s tiles
DT = D // P       # 8 d tiles
FT = FF // P      # 32 f tiles
LN_EPS = 1e-5
SCALE = 0.125     # 1/sqrt(64)

_CACHE = {}


def build():
    nc = bacc.Bacc("TRN2", target_bir_lowering=False, debug=False)

    # ---- DRAM I/O ----
    xt = nc.dram_tensor("xt", [D, S], F32, kind="ExternalInput").ap()        # x[b].T
    xr = nc.dram_tensor("xr", [S, D], F32, kind="ExternalInput").ap()        # x[b]
    wqkvT = nc.dram_tensor("wqkvT", [D, 3 * D], F32, kind="ExternalInput").ap()
    bqkv = nc.dram_tensor("bqkv", [3 * D], F32, kind="ExternalInput").ap()
    woutTbf = nc.dram_tensor("woutTbf", [D, D], BF16, kind="ExternalInput").ap()
    boutbf = nc.dram_tensor("boutbf", [1, D], BF16, kind="ExternalInput").ap()
    w1T16 = nc.dram_tensor("w1T16", [D, FF], F16, kind="ExternalInput").ap()
    b1 = nc.dram_tensor("b1", [FF], F32, kind="ExternalInput").ap()
    w2T16 = nc.dram_tensor("w2T16", [FF, D], F16, kind="ExternalInput").ap()
    b2_16 = nc.dram_tensor("b2_16", [1, D], F16, kind="ExternalInput").ap()
    ln1w = nc.dram_tensor("ln1w", [D], F32, kind="ExternalInput").ap()
    ln1b = nc.dram_tensor("ln1b", [D], F32, kind="ExternalInput").ap()
    ln2w = nc.dram_tensor("ln2w", [D], F32, kind="ExternalInput").ap()
    ln2b = nc.dram_tensor("ln2b", [D], F32, kind="ExternalInput").ap()
    maskrow = nc.dram_tensor("maskrow", [1, P], F32, kind="ExternalInput").ap()
    maskpart = nc.dram_tensor("maskpart", [P, 1], F32, kind="ExternalInput").ap()
    ones16 = nc.dram_tensor("ones16", [1, P], F16, kind="ExternalInput").ap()
    onesbf = nc.dram_tensor("onesbf", [1, P], BF16, kind="ExternalInput").ap()

    out_d = nc.dram_tensor("out", [S, D], F32, kind="ExternalOutput").ap()
    attn_d = nc.dram_tensor("attn", [H, S, S], F32, kind="ExternalOutput").ap()

    def bcast(ap_1d, parts=P):
        # DRAM AP replicated across `parts` partitions (step-0 partition dim)
        return bass.AP(tensor=ap_1d.tensor, offset=ap_1d.offset,
                       ap=[[0, parts]] + list(ap_1d.ap))

    with tile.TileContext(nc) as tc, ExitStack() as top:
        psum = top.enter_context(tc.tile_pool(name="psum", bufs=4, space="PSUM"))
        pstr = top.enter_context(tc.tile_pool(name="pstr", bufs=2, space="PSUM"))
        consts = top.enter_context(tc.tile_pool(name="consts", bufs=1))

        eps_t = consts.tile([P, 1], F32)
        nc.vector.memset(eps_t[:], LN_EPS)
        mask_sb = consts.tile([P, 1], F32)
        nc.sync.dma_start(mask_sb[:], maskpart[:])

        pers = top.enter_context(tc.tile_pool(name="pers", bufs=1))
        aoT = pers.tile([P, DT, S], BF16)    # attn_out^T (normalized)
        dramp = top.enter_context(tc.tile_pool(name="dramp", bufs=1, space="DRAM"))
        qkspill = dramp.tile([P, 10, S], F32R)  # Q/K tiles of pairs 3-7

        with ExitStack() as es_a:
            ga = es_a.enter_context(tc.tile_pool(name="ga", bufs=1))
            QKT = ga.tile([P, 16, S], F32R)      # Q^T tiles 0..7, K^T tiles 8..15
            Vp = ga.tile([P, NT, H, 65], BF16)   # V (s-major) + ones column
            bqk_part = ga.tile([P, 16], F32)     # Q,K bias (per-partition j)
            nc.sync.dma_start(bqk_part[:], bqkv[0:2048].rearrange("(t p) -> p t", p=P))
            bv_rep = ga.tile([P, D], F32)        # V bias replicated
            nc.sync.dma_start(bv_rep[:], bcast(bqkv[2048:3072]))
            mask_col = ga.tile([1, P], F32R)     # head_mask row (rank-1 lhsT)
            nc.sync.dma_start(mask_col[:], maskrow[:].bitcast(F32R))

            p2ab = es_a.enter_context(tc.tile_pool(name="p2ab", bufs=5))
            pzs = es_a.enter_context(tc.tile_pool(name="pzs", bufs=2))

            def emit_b_unit(hp, hh, lt):
                h = 2 * hp + hh
                b0 = 64 * hh
                ab = p2ab.tile([P, S], F32, tag="ab")
                zs = pzs.tile([P, 1], F32, tag="zs")
                ps = psum.tile([P, S], F32, tag="mm")
                for sc in range(2):
                    nc.tensor.matmul(
                        ps[:, sc * 512:(sc + 1) * 512],
                        QKT[b0:b0 + 64, hp, lt * 128:(lt + 1) * 128],
                        QKT[b0:b0 + 64, 8 + hp, sc * 512:(sc + 1) * 512],
                        start=True, stop=True, tile_position=(b0, 0))
                nc.scalar.activation(ab[:], ps[:], AF.Exp, scale=SCALE,
                                     accum_out=zs[:])
                nc.vector.reciprocal(out=zs[:], in_=zs[:])
                nc.vector.tensor_scalar(
                    out=ab[:], in0=ab[:], scalar1=zs[:],
                    scalar2=mask_sb[:], op0=ALU.mult, op1=ALU.mult)
                nc.sync.dma_start(attn_d[h, lt * 128:(lt + 1) * 128, :], ab[:])

            wqkvT_r = wqkvT.bitcast(F32R).rearrange("(kt p) j -> p kt j", p=P)
            xt_view = xt.bitcast(F32R).rearrange("(kt p) l -> p kt l", p=P)

            # ================= Phase 1: QKV projection =================
            with ExitStack() as s1:
                p1 = s1.enter_context(tc.tile_pool(name="p1", bufs=2))
                xt_r = s1.enter_context(tc.tile_pool(name="xtr", bufs=1)).tile(
                    [P, DT, S], F32R)
                for kt in range(DT):
                    nc.sync.dma_start(xt_r[:, kt, :], xt_view[:, kt, :])

                # ones column of Vp (broadcast over all dims)
                nc.sync.dma_start(
                    Vp[:, :, :, 64:65].rearrange("p st h c -> p (st h) c"),
                    bass.AP(tensor=onesbf.tensor, offset=onesbf.offset,
                            ap=[[0, P], [0, NT * H], [1, 1]]))

                # Q^T, K^T: out[j-tile, l]  (j = QKV output channel)
                for jc in (0, 2, 1, 3):            # Q/K interleaved so pair hp has
                                                   # both tiles early (phase-2 overlap)
                    w = p1.tile([P, DT, 512], F32R, tag="w")
                    nc.sync.dma_start(w[:], wqkvT_r[:, :, jc * 512:(jc + 1) * 512])
                    for jt4 in range(4):           # j-tile within chunk
                        jt = jc * 4 + jt4
                        for lc in range(2):
                            ps = psum.tile([P, 512], F32, tag="mm")
                            for kt in range(DT):
                                nc.tensor.matmul(
                                    ps[:], w[:, kt, jt4 * 128:(jt4 + 1) * 128],
                                    xt_r[:, kt, lc * 512:(lc + 1) * 512],
                                    start=(kt == 0), stop=(kt == DT - 1))
                            nc.vector.tensor_scalar(
                                out=QKT[:, jt, lc * 512:(lc + 1) * 512], in0=ps[:],
                                scalar1=bqk_part[:, jt:jt + 1], scalar2=None,
                                op0=ALU.add)
                # V: out[s-tile, vcol] (natural orientation) + bias, -> bf16
                for vc in range(2):
                    w = p1.tile([P, DT, 512], F32R, tag="w")
                    nc.sync.dma_start(
                        w[:], wqkvT_r[:, :, 2048 + vc * 512: 2048 + (vc + 1) * 512])
                    for st in range(NT):
                        ps = psum.tile([P, 512], F32, tag="mm")
                        for kt in range(DT):
                            nc.tensor.matmul(
                                ps[:], xt_r[:, kt, st * 128:(st + 1) * 128],
                                w[:, kt, :],
                                start=(kt == 0), stop=(kt == DT - 1))
                        nc.vector.tensor_tensor(
                            out=Vp[:, st, vc * 8:(vc + 1) * 8, 0:64],
                            in0=ps[:].rearrange("p (h d) -> p h d", h=8),
                            in1=bv_rep[:, vc * 512:(vc + 1) * 512].rearrange(
                                "p (h d) -> p h d", h=8),
                            op=ALU.add)

            # ================= Phase 2: attention =================
            with ExitStack() as s2:
                p2 = s2.enter_context(tc.tile_pool(name="p2", bufs=4))
                p2s = s2.enter_context(tc.tile_pool(name="p2s", bufs=2))

                expT = {}

                def scores_units(hp):
                    expT[hp] = {}
                    for hh in range(2 if P2_SCORES else 0):
                        b0 = 64 * hh
                        et = p2.tile([P, NT, S], BF16, tag="expT",
                                     name=f"expT{hp}_{hh}")
                        expT[hp][hh] = et
                        for st in range(NT):
                            def u(hp=hp, hh=hh, st=st, b0=b0, et=et):
                                ps = psum.tile([P, S], F32, tag="mm")
                                for lc in range(2):
                                    nc.tensor.matmul(
                                        ps[:, lc * 512:(lc + 1) * 512],
                                        QKT[b0:b0 + 64, 8 + hp,
                                            st * 128:(st + 1) * 128],
                                        QKT[b0:b0 + 64, hp,
                                            lc * 512:(lc + 1) * 512],
                                        start=True, stop=True,
                                        tile_position=(b0, 0))
                                nc.scalar.activation(
                                    et[:, st, :], ps[:], AF.Exp, scale=SCALE)
                            yield u

                def b_units(hp):
                    for hh in range(2 if P2_B else 0):
                        h = 2 * hp + hh
                        b0 = 64 * hh
                        for lt in range(NT):
                            def u(h=h, b0=b0, hp=hp, lt=lt):
                                ab = p2ab.tile([P, S], F32, tag="ab")
                                zs = p2s.tile([P, 1], F32, tag="zs")
                                ps = psum.tile([P, S], F32, tag="mm")
                                for sc in range(2):
                                    nc.tensor.matmul(
                                        ps[:, sc * 512:(sc + 1) * 512],
                                        QKT[b0:b0 + 64, hp,
                                            lt * 128:(lt + 1) * 128],
                                        QKT[b0:b0 + 64, 8 + hp,
                                            sc * 512:(sc + 1) * 512],
                                        start=True, stop=True,
                                        tile_position=(b0, 0))
                                nc.scalar.activation(
                                    ab[:], ps[:], AF.Exp, scale=SCALE,
                                    accum_out=zs[:])
                                nc.vector.reciprocal(out=zs[:], in_=zs[:])
                                nc.vector.tensor_scalar(
                                    out=ab[:], in0=ab[:], scalar1=zs[:],
                                    scalar2=mask_sb[:], op0=ALU.mult,
                                    op1=ALU.mult)
                                nc.sync.dma_start(
                                    attn_d[h, lt * 128:(lt + 1) * 128, :], ab[:])
                            yield u

                def do_av(hp):
                    for hh in range(2 if P2_AV else 0):
                        h = 2 * hp + hh
                        b0 = 64 * hh
                        pv = pstr.tile([P, S], F32, tag="ptw", name="pv")
                        for lc in range(2):
                            for st in range(NT):
                                nc.tensor.matmul(
                                    pv[0:65, lc * 512:(lc + 1) * 512],
                                    Vp[:, st, h, :],
                                    expT[hp][hh][:, st, lc * 512:(lc + 1) * 512],
                                    start=(st == 0), stop=(st == NT - 1))
                        pv_sb = p2s.tile([P, S], F32, tag="pvsb")
                        nc.vector.tensor_copy(pv_sb[0:65, :], pv[0:65, :])
                        rz = p2s.tile([1, S], F32R, tag="rz")
                        with nc.allow_low_precision(reason="f32r is bitwise f32"):
                            nc.vector.reciprocal(out=rz[:], in_=pv_sb[64:65, :])
                        prep = psum.tile([P, S], F32, tag="mm", name="prep")
                        for lc in range(2):
                            nc.tensor.matmul(prep[:, lc * 512:(lc + 1) * 512],
                                             mask_col[:],
                                             rz[:, lc * 512:(lc + 1) * 512],
                                             start=True, stop=True)
                        nc.vector.tensor_tensor(
                            out=aoT[b0:b0 + 64, hp, :], in0=pv_sb[0:64, :],
                            in1=prep[b0:b0 + 64, :], op=ALU.mult)
                    del expT[hp]

                # fine-grained pipeline: alternate pair t+1's score/exp units
                # with pair t's DMA-paced attn-output units so ACT never
                # starves; attn@V trails one pair behind
                from itertools import zip_longest
                for hp in range(H // 2):
                    su = list(scores_units(hp))
                    bu = list(b_units(hp - 1)) if 1 <= hp - 1 < 3 else []
                    for a, b_ in zip_longest(su, bu):
                        if a: a()
                        if b_: b_()
                    if hp > 0:
                        do_av(hp - 1)
                do_av(H // 2 - 1)

        # ============ Phase 3: out_proj + residual + LN1 (bf16 mm) ============
        with ExitStack() as es_d:
            gd = es_d.enter_context(tc.tile_pool(name="gd", bufs=1))
            hT = gd.tile([P, FT, S], F16)
            x1 = gd.tile([P, NT, D], F32)    # LN1 output (l-major)
            b1_part = gd.tile([P, FT], F32)
            nc.sync.dma_start(b1_part[:], b1[:].rearrange("(t p) -> p t", p=P))

            with ExitStack() as s3:
                c3 = s3.enter_context(tc.tile_pool(name="c3", bufs=1))
                wo = c3.tile([P, DT, D], BF16)
                nc.sync.dma_start(wo[:], woutTbf.rearrange("(kt p) n -> p kt n", p=P))
                boutrow = c3.tile([1, D], BF16)
                nc.sync.dma_start(boutrow[:], boutbf[:])
                ones_colbf = c3.tile([1, P], BF16)
                nc.sync.dma_start(ones_colbf[:], onesbf[:])
                ln1w_rep = c3.tile([P, D], F32)
                nc.sync.dma_start(ln1w_rep[:], bcast(ln1w))
                ln1b_rep = c3.tile([P, D], F32)
                nc.sync.dma_start(ln1b_rep[:], bcast(ln1b))
                ident = c3.tile([P, P], F32)
                make_identity(nc, ident[:])
                x1T = s3.enter_context(tc.tile_pool(name="x1Tp", bufs=1)).tile(
                    [P, DT, S], F16)
                p3 = s3.enter_context(tc.tile_pool(name="p3", bufs=2))

                for lt in range(NT):
                    xrt = p3.tile([P, D], F32, tag="xrt")
                    nc.sync.dma_start(xrt[:], xr[lt * 128:(lt + 1) * 128, :])
                    x2p = p3.tile([P, D], F32, tag="x2p")
                    for dc in range(2):
                        ps = psum.tile([P, 512], F32, tag="mm")
                        for kt in range(DT):
                            nc.tensor.matmul(
                                ps[:], aoT[:, kt, lt * 128:(lt + 1) * 128],
                                wo[:, kt, dc * 512:(dc + 1) * 512],
                                start=(kt == 0), stop=False)
                        nc.tensor.matmul(ps[:], ones_colbf[:],
                                         boutrow[:, dc * 512:(dc + 1) * 512],
                                         start=False, stop=True)
                        nc.vector.tensor_tensor(
                            out=x2p[:, dc * 512:(dc + 1) * 512], in0=ps[:],
                            in1=xrt[:, dc * 512:(dc + 1) * 512], op=ALU.add)
                    # LN1
                    stats = p3.tile([P, 2, nc.vector.BN_STATS_DIM], F32, tag="st")
                    for g in range(2):
                        nc.vector.bn_stats(stats[:, g, :],
                                           x2p[:, g * 512:(g + 1) * 512])
                    mv = p3.tile([P, nc.vector.BN_AGGR_DIM], F32, tag="mv")
                    nc.vector.bn_aggr(mv[:], stats[:])
                    rstd = p3.tile([P, 1], F32, tag="rstd")
                    nc.scalar.activation(rstd[:], mv[:, 1:2], AF.Sqrt,
                                         bias=eps_t[:], scale=1.0)
                    nc.vector.reciprocal(out=rstd[:], in_=rstd[:])
                    nc.vector.scalar_tensor_tensor(
                        out=x2p[:], in0=x2p[:], scalar=mv[:, 0:1], in1=ln1w_rep[:],
                        op0=ALU.subtract, op1=ALU.mult)
                    nc.vector.scalar_tensor_tensor(
                        out=x1[:, lt, :], in0=x2p[:], scalar=rstd[:],
                        in1=ln1b_rep[:], op0=ALU.mult, op1=ALU.add)
                    # transpose x1 tile -> x1T (f16) for FFN: 8 PE transposes
                    # into one wide psum tile, one wide copy out
                    pt = pstr.tile([P, S], F32, tag="ptw")
                    for dc in range(DT):
                        nc.tensor.transpose(pt[:, dc * 128:(dc + 1) * 128],
                                            x1[:, lt, dc * 128:(dc + 1) * 128],
                                            ident[:])
                    nc.vector.tensor_copy(
                        x1T[:, :, lt * 128:(lt + 1) * 128],
                        pt[:].rearrange("p (dt c) -> p dt c", dt=DT))

                # ================= Phase 4: FFN1 (f16) =================
                p4 = s3.enter_context(tc.tile_pool(name="p4", bufs=3))
                w1v = w1T16.rearrange("(kt p) f -> p kt f", p=P)
                for ft in range(FT):
                    w1s = p4.tile([P, DT, P], F16, tag="w1s")
                    nc.sync.dma_start(w1s[:], w1v[:, :, ft * 128:(ft + 1) * 128])
                    for lc in range(2):
                        ps = psum.tile([P, 512], F32, tag="mm")
                        for kt in range(DT):
                            nc.tensor.matmul(
                                ps[:], w1s[:, kt, :],
                                x1T[:, kt, lc * 512:(lc + 1) * 512],
                                start=(kt == 0), stop=(kt == DT - 1))
                        nc.scalar.activation(
                            hT[:, ft, lc * 512:(lc + 1) * 512], ps[:], AF.Relu,
                            bias=b1_part[:, ft:ft + 1], scale=1.0)

            # ================= Phase 5: FFN2 (f16) + LN2 + out =================
            with ExitStack() as s5:
                c5 = s5.enter_context(tc.tile_pool(name="c5", bufs=1))
                w2 = c5.tile([P, FT, D], F16)
                w2v = w2T16.rearrange("(kt p) n -> p kt n", p=P)
                for kt in range(FT):
                    nc.sync.dma_start(w2[:, kt, :], w2v[:, kt, :])
                b2row = c5.tile([1, D], F16)
                nc.sync.dma_start(b2row[:], b2_16[:])
                ones_col16 = c5.tile([1, P], F16)
                nc.sync.dma_start(ones_col16[:], ones16[:])
                ln2w_rep = c5.tile([P, D], F32)
                nc.sync.dma_start(ln2w_rep[:], bcast(ln2w))
                ln2b_rep = c5.tile([P, D], F32)
                nc.sync.dma_start(ln2b_rep[:], bcast(ln2b))
                p5 = s5.enter_context(tc.tile_pool(name="p5", bufs=2))

                for lt in range(NT):
                    x2p = p5.tile([P, D], F32, tag="x2p2")
                    for dc in range(2):
                        ps = psum.tile([P, 512], F32, tag="mm")
                        for kt in range(FT):
                            nc.tensor.matmul(
                                ps[:], hT[:, kt, lt * 128:(lt + 1) * 128],
                                w2[:, kt, dc * 512:(dc + 1) * 512],
                                start=(kt == 0), stop=False)
                        nc.tensor.matmul(ps[:], ones_col16[:],
                                         b2row[:, dc * 512:(dc + 1) * 512],
                                         start=False, stop=True)
                        nc.vector.tensor_tensor(
                            out=x2p[:, dc * 512:(dc + 1) * 512], in0=ps[:],
                            in1=x1[:, lt, dc * 512:(dc + 1) * 512], op=ALU.add)
                    stats = p5.tile([P, 2, nc.vector.BN_STATS_DIM], F32, tag="st2")
                    for g in range(2):
                        nc.vector.bn_stats(stats[:, g, :],
                                           x2p[:, g * 512:(g + 1) * 512])
                    mv = p5.tile([P, nc.vector.BN_AGGR_DIM], F32, tag="mv2")
                    nc.vector.bn_aggr(mv[:], stats[:])
                    rstd = p5.tile([P, 1], F32, tag="rstd2")
                    nc.scalar.activation(rstd[:], mv[:, 1:2], AF.Sqrt,
                                         bias=eps_t[:], scale=1.0)
                    nc.vector.reciprocal(out=rstd[:], in_=rstd[:])
                    nc.vector.scalar_tensor_tensor(
                        out=x2p[:], in0=x2p[:], scalar=mv[:, 0:1], in1=ln2w_rep[:],
                        op0=ALU.subtract, op1=ALU.mult)
                    ot = p5.tile([P, D], F32, tag="ot")
                    nc.vector.scalar_tensor_tensor(
                        out=ot[:], in0=x2p[:], scalar=rstd[:], in1=ln2b_rep[:],
                        op0=ALU.mult, op1=ALU.add)
                    nc.sync.dma_start(out_d[lt * 128:(lt + 1) * 128, :], ot[:])

    nc.compile()
    return nc


def make_in_maps(x, in_proj_w, in_proj_b, out_proj_w, out_proj_b,
                 ln1_w, ln1_b, ln2_w, ln2_b,
                 ffn_w1, ffn_b1, ffn_w2, ffn_b2, head_mask):
    f32 = np.float32
    bf16 = ml_dtypes.bfloat16
    shared = dict(
        wqkvT=np.ascontiguousarray(np.asarray(in_proj_w, f32).T),
        bqkv=np.asarray(in_proj_b, f32),
        woutTbf=np.ascontiguousarray(np.asarray(out_proj_w, f32).T).astype(bf16),
        boutbf=np.asarray(out_proj_b, f32).reshape(1, D).astype(bf16),
        w1T16=np.ascontiguousarray(np.asarray(ffn_w1, f32).T).astype(np.float16),
        b1=np.asarray(ffn_b1, f32),
        w2T16=np.ascontiguousarray(np.asarray(ffn_w2, f32).T).astype(np.float16),
        b2_16=np.asarray(ffn_b2, f32).reshape(1, D).astype(np.float16),
        ln1w=np.asarray(ln1_w, f32), ln1b=np.asarray(ln1_b, f32),
        ln2w=np.asarray(ln2_w, f32), ln2b=np.asarray(ln2_b, f32),
        maskrow=np.full((1, P), float(np.asarray(head_mask)), f32),
        maskpart=np.full((P, 1), float(np.asarray(head_mask)), f32),
        ones16=np.ones((1, P), np.float16),
        onesbf=np.ones((1, P), f32).astype(bf16),
    )
    x = np.asarray(x, f32)
    in_maps = []
    for b in range(B):
        m = dict(shared)
        m["xr"] = np.ascontiguousarray(x[b])
        m["xt"] = np.ascontiguousarray(x[b].T)
        in_maps.append(m)
    return in_maps


def kernel(**inputs):
    if "nc" not in _CACHE:
        _CACHE["nc"] = build()
    nc = _CACHE["nc"]
    in_maps = make_in_maps(**inputs)
    res = bass_utils.run_bass_kernel_spmd(nc, in_maps, core_ids=list(range(B)))
    out = np.stack([r["out"] for r in res.results])
    attn = np.stack([r["attn"] for r in res.results])
    return out, attn


# revision 53
# speedup vs baseline: 1.4924x; 1.4924x over previous
"""TRN2 Bass kernel for nn_AttentionAttributionTransformerEncoderLayer.

Data-parallel over batch: 8 batch elements -> 8 NeuronCores, no collectives.
Per core: full post-LN transformer encoder layer on [S=1024, D=1024] plus the
[H=16, S, S] attention-weight output.

Matmul dtypes: float32r (full-rate, ~1.4e-4 rel err) for QKV / scores;
bf16 for exp/V tiles feeding attn@V and for out_proj; fp16 for both FFN
matmuls. PSUM accumulation is always f32.
"""
import sys

sys.path.insert(0, "/opt/trn_rl_repo")

from contextlib import ExitStack

import ml_dtypes
import numpy as np

import concourse.bass as bass
import concourse.tile as tile
import concourse.mybir as mybir
from concourse import bacc, bass_utils
from concourse.masks import make_identity

F32 = mybir.dt.float32
F32R = mybir.dt.float32r
F16 = mybir.dt.float16
BF16 = mybir.dt.bfloat16
AF = mybir.ActivationFunctionType
ALU = mybir.AluOpType

B, S, D = 8, 1024, 1024
H, DH = 16, 64
FF = 4096
P = 128
NT = S // P       # 8 l/s tiles
DT = D // P       # 8 d tiles
FT = FF // P      # 32 f tiles
LN_EPS = 1e-5
SCALE = 0.125     # 1/sqrt(64)

_CACHE = {}


def build():
    nc = bacc.Bacc("TRN2", target_bir_lowering=False, debug=False)

    # ---- DRAM I/O ----
    xt = nc.dram_tensor("xt", [D, S], F32, kind="ExternalInput").ap()        # x[b].T
    xr = nc.dram_tensor("xr", [S, D], F32, kind="ExternalInput").ap()        # x[b]
    wqkvT = nc.dram_tensor("wqkvT", [D, 3 * D], F32, kind="ExternalInput").ap()
    bqkv = nc.dram_tensor("bqkv", [3 * D], F32, kind="ExternalInput").ap()
    woutTbf = nc.dram_tensor("woutTbf", [D, D], BF16, kind="ExternalInput").ap()
    boutbf = nc.dram_tensor("boutbf", [1, D], BF16, kind="ExternalInput").ap()
    w1T16 = nc.dram_tensor("w1T16", [D, FF], F16, kind="ExternalInput").ap()
    b1 = nc.dram_tensor("b1", [FF], F32, kind="ExternalInput").ap()
    w2T16 = nc.dram_tensor("w2T16", [FF, D], F16, kind="ExternalInput").ap()
    b2_16 = nc.dram_tensor("b2_16", [1, D], F16, kind="ExternalInput").ap()
    ln1w = nc.dram_tensor("ln1w", [D], F32, kind="ExternalInput").ap()
    ln1b = nc.dram_tensor("ln1b", [D], F32, kind="ExternalInput").ap()
    ln2w = nc.dram_tensor("ln2w", [D], F32, kind="ExternalInput").ap()
    ln2b = nc.dram_tensor("ln2b", [D], F32, kind="ExternalInput").ap()
    maskrow = nc.dram_tensor("maskrow", [1, P], F32, kind="ExternalInput").ap()
    maskpart = nc.dram_tensor("maskpart", [P, 1], F32, kind="ExternalInput").ap()
    ones16 = nc.dram_tensor("ones16", [1, P], F16, kind="ExternalInput").ap()
    onesbf = nc.dram_tensor("onesbf", [1, P], BF16, kind="ExternalInput").ap()

    out_d = nc.dram_tensor("out", [S, D], F32, kind="ExternalOutput").ap()
    attn_d = nc.dram_tensor("attn", [H, S, S], F32, kind="ExternalOutput").ap()

    def bcast(ap_1d, parts=P):
        # DRAM AP replicated across `parts` partitions (step-0 partition dim)
        return bass.AP(tensor=ap_1d.tensor, offset=ap_1d.offset,
                       ap=[[0, parts]] + list(ap_1d.ap))

    with tile.TileContext(nc) as tc, ExitStack() as top:
        psum = top.enter_context(tc.tile_pool(name="psum", bufs=4, space="PSUM"))
        pstr = top.enter_context(tc.tile_pool(name="pstr", bufs=2, space="PSUM"))
        consts = top.enter_context(tc.tile_pool(name="consts", bufs=1))

        eps_t = consts.tile([P, 1], F32)
        nc.vector.memset(eps_t[:], LN_EPS)
        mask_sb = consts.tile([P, 1], F32)
        nc.sync.dma_start(mask_sb[:], maskpart[:])

        pers = top.enter_context(tc.tile_pool(name="pers", bufs=1))
        aoT = pers.tile([P, DT, S], BF16)    # attn_out^T (normalized)
        dramp = top.enter_context(tc.tile_pool(name="dramp", bufs=1, space="DRAM"))
        qkspill = dramp.tile([P, 10, S], F32R)  # Q/K tiles of pairs 3-7

        with ExitStack() as es_a:
            ga = es_a.enter_context(tc.tile_pool(name="ga", bufs=1))
            QKT = ga.tile([P, 16, S], F32R)      # Q^T tiles 0..7, K^T tiles 8..15
            Vp = ga.tile([P, NT, H, 65], BF16)   # V (s-major) + ones column
            bqk_part = ga.tile([P, 16], F32)     # Q,K bias (per-partition j)
            nc.sync.dma_start(bqk_part[:], bqkv[0:2048].rearrange("(t p) -> p t", p=P))
            bv_rep = ga.tile([P, D], F32)        # V bias replicated
            nc.sync.dma_start(bv_rep[:], bcast(bqkv[2048:3072]))
            mask_col = ga.tile([1, P], F32R)     # head_mask row (rank-1 lhsT)
            nc.sync.dma_start(mask_col[:], maskrow[:].bitcast(F32R))

            p2ab = es_a.enter_context(tc.tile_pool(name="p2ab", bufs=5))
            pzs = es_a.enter_context(tc.tile_pool(name="pzs", bufs=2))

            def emit_b_unit(hp, hh, lt):
                h = 2 * hp + hh
                b0 = 64 * hh
                ab = p2ab.tile([P, S], F32, tag="ab")
                zs = pzs.tile([P, 1], F32, tag="zs")
                ps = psum.tile([P, S], F32, tag="mm")
                for sc in range(2):
                    nc.tensor.matmul(
                        ps[:, sc * 512:(sc + 1) * 512],
                        QKT[b0:b0 + 64, hp, lt * 128:(lt + 1) * 128],
                        QKT[b0:b0 + 64, 8 + hp, sc * 512:(sc + 1) * 512],
                        start=True, stop=True, tile_position=(b0, 0))
                nc.scalar.activation(ab[:], ps[:], AF.Exp, scale=SCALE,
                                     accum_out=zs[:])
                nc.vector.reciprocal(out=zs[:], in_=zs[:])
                nc.vector.tensor_scalar(
                    out=ab[:], in0=ab[:], scalar1=zs[:],
                    scalar2=mask_sb[:], op0=ALU.mult, op1=ALU.mult)
                nc.sync.dma_start(attn_d[h, lt * 128:(lt + 1) * 128, :], ab[:])

            wqkvT_r = wqkvT.bitcast(F32R).rearrange("(kt p) j -> p kt j", p=P)
            xt_view = xt.bitcast(F32R).rearrange("(kt p) l -> p kt l", p=P)

            # ================= Phase 1: QKV projection =================
            with ExitStack() as s1:
                p1 = s1.enter_context(tc.tile_pool(name="p1", bufs=2))
                xt_r = s1.enter_context(tc.tile_pool(name="xtr", bufs=1)).tile(
                    [P, DT, S], F32R)
                for kt in range(DT):
                    nc.sync.dma_start(xt_r[:, kt, :], xt_view[:, kt, :])

                # ones column of Vp (broadcast over all dims)
                nc.sync.dma_start(
                    Vp[:, :, :, 64:65].rearrange("p st h c -> p (st h) c"),
                    bass.AP(tensor=onesbf.tensor, offset=onesbf.offset,
                            ap=[[0, P], [0, NT * H], [1, 1]]))

                # Q^T, K^T: out[j-tile, l]  (j = QKV output channel)
                for jc in (0, 2, 1, 3):            # Q/K interleaved so pair hp has
                                                   # both tiles early (phase-2 overlap)
                    w = p1.tile([P, DT, 512], F32R, tag="w")
                    nc.sync.dma_start(w[:], wqkvT_r[:, :, jc * 512:(jc + 1) * 512])
                    for jt4 in range(4):           # j-tile within chunk
                        jt = jc * 4 + jt4
                        for lc in range(2):
                            ps = psum.tile([P, 512], F32, tag="mm")
                            for kt in range(DT):
                                nc.tensor.matmul(
                                    ps[:], w[:, kt, jt4 * 128:(jt4 + 1) * 128],
                                    xt_r[:, kt, lc * 512:(lc + 1) * 512],
                                    start=(kt == 0), stop=(kt == DT - 1))
                            nc.vector.tensor_scalar(
                                out=QKT[:, jt, lc * 512:(lc + 1) * 512], in0=ps[:],
                                scalar1=bqk_part[:, jt:jt + 1], scalar2=None,
                                op0=ALU.add)
                # V: out[s-tile, vcol] (natural orientation) + bias, -> bf16
                for vc in range(2):
                    w = p1.tile([P, DT, 512], F32R, tag="w")
                    nc.sync.dma_start(
                        w[:], wqkvT_r[:, :, 2048 + vc * 512: 2048 + (vc + 1) * 512])
                    for st in range(NT):
                        ps = psum.tile([P, 512], F32, tag="mm")
                        for kt in range(DT):
                            nc.tensor.matmul(
                                ps[:], xt_r[:, kt, st * 128:(st + 1) * 128],
                                w[:, kt, :],
                                start=(kt == 0), stop=(kt == DT - 1))
                        nc.vector.tensor_tensor(
                            out=Vp[:, st, vc * 8:(vc + 1) * 8, 0:64],
                            in0=ps[:].rearrange("p (h d) -> p h d", h=8),
                            in1=bv_rep[:, vc * 512:(vc + 1) * 512].rearrange(
                                "p (h d) -> p h d", h=8),
                            op=ALU.add)

            # ================= Phase 2: attention =================
            with ExitStack() as s2:
                p2 = s2.enter_context(tc.tile_pool(name="p2", bufs=4))
                p2s = s2.enter_context(tc.tile_pool(name="p2s", bufs=2))

                expT = {}

                def scores_units(hp):
                    expT[hp] = {}
                    for hh in range(2 if P2_SCORES else 0):
                        b0 = 64 * hh
                        et = p2.tile([P, NT, S], BF16, tag="expT",
                                     name=f"expT{hp}_{hh}")
                        expT[hp][hh] = et
                        for st in range(NT):
                            def u(hp=hp, hh=hh, st=st, b0=b0, et=et):
                                ps = psum.tile([P, S], F32, tag="mm")
                                for lc in range(2):
                                    nc.tensor.matmul(
                                        ps[:, lc * 512:(lc + 1) * 512],
                                        QKT[b0:b0 + 64, 8 + hp,
                                            st * 128:(st + 1) * 128],
                                        QKT[b0:b0 + 64, hp,
                                            lc * 512:(lc + 1) * 512],
                                        start=True, stop=True,
                                        tile_position=(b0, 0))
                                nc.scalar.activation(
                                    et[:, st, :], ps[:], AF.Exp, scale=SCALE)
                            yield u

                def b_units(hp):
                    for hh in range(2 if P2_B else 0):
                        h = 2 * hp + hh
                        b0 = 64 * hh
                        for lt in range(NT):
                            def u(h=h, b0=b0, hp=hp, lt=lt):
                                ab = p2ab.tile([P, S], F32, tag="ab")
                                zs = p2s.tile([P, 1], F32, tag="zs")
                                ps = psum.tile([P, S], F32, tag="mm")
                                for sc in range(2):
                                    nc.tensor.matmul(
                                        ps[:, sc * 512:(sc + 1) * 512],
                                        QKT[b0:b0 + 64, hp,
                                            lt * 128:(lt + 1) * 128],
                                        QKT[b0:b0 + 64, 8 + hp,
                                            sc * 512:(sc + 1) * 512],
                                        start=True, stop=True,
                                        tile_position=(b0, 0))
                                nc.scalar.activation(
                                    ab[:], ps[:], AF.Exp, scale=SCALE,
                                    accum_out=zs[:])
                                nc.vector.reciprocal(out=zs[:], in_=zs[:])
                                nc.vector.tensor_scalar(
                                    out=ab[:], in0=ab[:], scalar1=zs[:],
                                    scalar2=mask_sb[:], op0=ALU.mult,
                                    op1=ALU.mult)
                                nc.sync.dma_start(
                                    attn_d[h, lt * 128:(lt + 1) * 128, :], ab[:])
                            yield u

                def do_av(hp):
                    for hh in range(2 if P2_AV else 0):
                        h = 2 * hp + hh
                        b0 = 64 * hh
                        pv = pstr.tile([P, S], F32, tag="ptw", name="pv")
                        for lc in range(2):
                            for st in range(NT):
                                nc.tensor.matmul(
                                    pv[0:65, lc * 512:(lc + 1) * 512],
                                    Vp[:, st, h, :],
                                    expT[hp][hh][:, st, lc * 512:(lc + 1) * 512],
                                    start=(st == 0), stop=(st == NT - 1))
                        pv_sb = p2s.tile([P, S], F32, tag="pvsb")
                        nc.vector.tensor_copy(pv_sb[0:65, :], pv[0:65, :])
                        rz = p2s.tile([1, S], F32R, tag="rz")
                        with nc.allow_low_precision(reason="f32r is bitwise f32"):
                            nc.vector.reciprocal(out=rz[:], in_=pv_sb[64:65, :])
                        prep = psum.tile([P, S], F32, tag="mm", name="prep")
                        for lc in range(2):
                            nc.tensor.matmul(prep[:, lc * 512:(lc + 1) * 512],
                                             mask_col[:],
                                             rz[:, lc * 512:(lc + 1) * 512],
                                             start=True, stop=True)
                        nc.vector.tensor_tensor(
                            out=aoT[b0:b0 + 64, hp, :], in0=pv_sb[0:64, :],
                            in1=prep[b0:b0 + 64, :], op=ALU.mult)
                    del expT[hp]

                # fine-grained pipeline: alternate pair t+1's score/exp units
                # with pair t's DMA-paced attn-output units so ACT never
                # starves; attn@V trails one pair behind
                from itertools import zip_longest
                for hp in range(H // 2):
                    su = list(scores_units(hp))
                    bu = list(b_units(hp - 1)) if 1 <= hp - 1 < 3 else []
                    for a, b_ in zip_longest(su, bu):
                        if a: a()
                        if b_: b_()
                    if hp > 0:
                        do_av(hp - 1)
                do_av(H // 2 - 1)

        # ============ Phase 3: out_proj + residual + LN1 (bf16 mm) ============
        with ExitStack() as es_d:
            gd = es_d.enter_context(tc.tile_pool(name="gd", bufs=1))
            hT = gd.tile([P, FT, S], F16)
            x1 = gd.tile([P, NT, D], F32)    # LN1 output (l-major)
            b1_part = gd.tile([P, FT], F32)
            nc.sync.dma_start(b1_part[:], b1[:].rearrange("(t p) -> p t", p=P))

            with ExitStack() as s3:
                c3 = s3.enter_context(tc.tile_pool(name="c3", bufs=1))
                wo = c3.tile([P, DT, D], BF16)
                nc.sync.dma_start(wo[:], woutTbf.rearrange("(kt p) n -> p kt n", p=P))
                boutrow = c3.tile([1, D], BF16)
                nc.sync.dma_start(boutrow[:], boutbf[:])
                ones_colbf = c3.tile([1, P], BF16)
                nc.sync.dma_start(ones_colbf[:], onesbf[:])
                ln1w_rep = c3.tile([P, D], F32)
                nc.sync.dma_start(ln1w_rep[:], bcast(ln1w))
                ln1b_rep = c3.tile([P, D], F32)
                nc.sync.dma_start(ln1b_rep[:], bcast(ln1b))
                ident = c3.tile([P, P], F32)
                make_identity(nc, ident[:])
                x1T = s3.enter_context(tc.tile_pool(name="x1Tp", bufs=1)).tile(
                    [P, DT, S], F16)
                p3 = s3.enter_context(tc.tile_pool(name="p3", bufs=2))

                for lt in range(NT):
                    xrt = p3.tile([P, D], F32, tag="xrt")
                    nc.sync.dma_start(xrt[:], xr[lt * 128:(lt + 1) * 128, :])
                    x2p = p3.tile([P, D], F32, tag="x2p")
                    for dc in range(2):
                        ps = psum.tile([P, 512], F32, tag="mm")
                        for kt in range(DT):
                            nc.tensor.matmul(
                                ps[:], aoT[:, kt, lt * 128:(lt + 1) * 128],
                                wo[:, kt, dc * 512:(dc + 1) * 512],
                                start=(kt == 0), stop=False)
                        nc.tensor.matmul(ps[:], ones_colbf[:],
                                         boutrow[:, dc * 512:(dc + 1) * 512],
                                         start=False, stop=True)
                        nc.vector.tensor_tensor(
                            out=x2p[:, dc * 512:(dc + 1) * 512], in0=ps[:],
                            in1=xrt[:, dc * 512:(dc + 1) * 512], op=ALU.add)
                    # LN1
                    stats = p3.tile([P, 2, nc.vector.BN_STATS_DIM], F32, tag="st")
                    for g in range(2):
                        nc.vector.bn_stats(stats[:, g, :],
                                           x2p[:, g * 512:(g + 1) * 512])
                    mv = p3.tile([P, nc.vector.BN_AGGR_DIM], F32, tag="mv")
                    nc.vector.bn_aggr(mv[:], stats[:])
                    rstd = p3.tile([P, 1], F32, tag="rstd")
                    nc.scalar.activation(rstd[:], mv[:, 1:2], AF.Sqrt,
                                         bias=eps_t[:], scale=1.0)
                    nc.vector.reciprocal(out=rstd[:], in_=rstd[:])
                    nc.vector.scalar_tensor_tensor(
                        out=x2p[:], in0=x2p[:], scalar=mv[:, 0:1], in1=ln1w_rep[:],
                        op0=ALU.subtract, op1=ALU.mult)
                    nc.vector.scalar_tensor_tensor(
                        out=x1[:, lt, :], in0=x2p[:], scalar=rstd[:],
                        in1=ln1b_rep[:], op0=ALU.mult, op1=ALU.add)
                    # transpose x1 tile -> x1T (f16) for FFN: 8 PE transposes
                    # into one wide psum tile, one wide copy out
                    pt = pstr.tile([P, S], F32, tag="ptw")
                    for dc in range(DT):
                        nc.tensor.transpose(pt[:, dc * 128:(dc + 1) * 128],
                                            x1[:, lt, dc * 128:(dc + 1) * 128],
                                            ident[:])
                    nc.vector.tensor_copy(
                        x1T[:, :, lt * 128:(lt + 1) * 128],
                        pt[:].rearrange("p (dt c) -> p dt c", dt=DT))

                # ================= Phase 4: FFN1 (f16) =================
                p4 = s3.enter_context(tc.tile_pool(name="p4", bufs=3))
                w1v = w1T16.rearrange("(kt p) f -> p kt f", p=P)
                for ft in range(FT):
                    w1s = p4.tile([P, DT, P], F16, tag="w1s")
                    nc.sync.dma_start(w1s[:], w1v[:, :, ft * 128:(ft + 1) * 128])
                    for lc in range(2):
                        ps = psum.tile([P, 512], F32, tag="mm")
                        for kt in range(DT):
                            nc.tensor.matmul(
                                ps[:], w1s[:, kt, :],
                                x1T[:, kt, lc * 512:(lc + 1) * 512],
                                start=(kt == 0), stop=(kt == DT - 1))
                        nc.scalar.activation(
                            hT[:, ft, lc * 512:(lc + 1) * 512], ps[:], AF.Relu,
                            bias=b1_part[:, ft:ft + 1], scale=1.0)

            # ================= Phase 5: FFN2 (f16) + LN2 + out =================
            with ExitStack() as s5:
                c5 = s5.enter_context(tc.tile_pool(name="c5", bufs=1))
                w2 = c5.tile([P, FT, D], F16)
                w2v = w2T16.rearrange("(kt p) n -> p kt n", p=P)
                for kt in range(FT):
                    nc.sync.dma_start(w2[:, kt, :], w2v[:, kt, :])
                b2row = c5.tile([1, D], F16)
                nc.sync.dma_start(b2row[:], b2_16[:])
                ones_col16 = c5.tile([1, P], F16)
                nc.sync.dma_start(ones_col16[:], ones16[:])
                ln2w_rep = c5.tile([P, D], F32)
                nc.sync.dma_start(ln2w_rep[:], bcast(ln2w))
                ln2b_rep = c5.tile([P, D], F32)
                nc.sync.dma_start(ln2b_rep[:], bcast(ln2b))
                p5 = s5.enter_context(tc.tile_pool(name="p5", bufs=2))

                for lt in range(NT):
                    x2p = p5.tile([P, D], F32, tag="x2p2")
                    for dc in range(2):
                        ps = psum.tile([P, 512], F32, tag="mm")
                        for kt in range(FT):
                            nc.tensor.matmul(
                                ps[:], hT[:, kt, lt * 128:(lt + 1) * 128],
                                w2[:, kt, dc * 512:(dc + 1) * 512],
                                start=(kt == 0), stop=False)
                        nc.tensor.matmul(ps[:], ones_col16[:],
                                         b2row[:, dc * 512:(dc + 1) * 512],
                                         start=False, stop=True)
                        nc.vector.tensor_tensor(
                            out=x2p[:, dc * 512:(dc + 1) * 512], in0=ps[:],
                            in1=x1[:, lt, dc * 512:(dc + 1) * 512], op=ALU.add)
                    stats = p5.tile([P, 2, nc.vector.BN_STATS_DIM], F32, tag="st2")
                    for g in range(2):
                        nc.vector.bn_stats(stats[:, g, :],
                                           x2p[:, g * 512:(g + 1) * 512])
                    mv = p5.tile([P, nc.vector.BN_AGGR_DIM], F32, tag="mv2")
                    nc.vector.bn_aggr(mv[:], stats[:])
                    rstd = p5.tile([P, 1], F32, tag="rstd2")
                    nc.scalar.activation(rstd[:], mv[:, 1:2], AF.Sqrt,
                                         bias=eps_t[:], scale=1.0)
                    nc.vector.reciprocal(out=rstd[:], in_=rstd[:])
                    nc.vector.scalar_tensor_tensor(
                        out=x2p[:], in0=x2p[:], scalar=mv[:, 0:1], in1=ln2w_rep[:],
                        op0=ALU.subtract, op1=ALU.mult)
                    ot = p5.tile([P, D], F32, tag="ot")
                    nc.vector.scalar_tensor_tensor(
                        out=ot[:], in0=x2p[:], scalar=rstd[:], in1=ln2b_rep[:],
                        op0=ALU.mult, op1=ALU.add)
                    nc.sync.dma_start(out_d[lt * 128:(lt + 1) * 128, :], ot[:])

    nc.compile()
    return nc


def make_in_maps(x, in_proj_w, in_proj_b, out_proj_w, out_proj_b,
                 ln1_w, ln1_b, ln2_w, ln2_b,
                 ffn_w1, ffn_b1, ffn_w2, ffn_b2, head_mask):
    f32 = np.float32
    bf16 = ml_dtypes.bfloat16
    shared = dict(
        wqkvT=np.ascontiguousarray(np.asarray(in_proj_w, f32).T),
        bqkv=np.asarray(in_proj_b, f32),
        woutTbf=np.ascontiguousarray(np.asarray(out_proj_w, f32).T).astype(bf16),
        boutbf=np.asarray(out_proj_b, f32).reshape(1, D).astype(bf16),
        w1T16=np.ascontiguousarray(np.asarray(ffn_w1, f32).T).astype(np.float16),
        b1=np.asarray(ffn_b1, f32),
        w2T16=np.ascontiguousarray(np.asarray(ffn_w2, f32).T).astype(np.float16),
        b2_16=np.asarray(ffn_b2, f32).reshape(1, D).astype(np.float16),
        ln1w=np.asarray(ln1_w, f32), ln1b=np.asarray(ln1_b, f32),
        ln2w=np.asarray(ln2_w, f32), ln2b=np.asarray(ln2_b, f32),
        maskrow=np.full((1, P), float(np.asarray(head_mask)), f32),
        maskpart=np.full((P, 1), float(np.asarray(head_mask)), f32),
        ones16=np.ones((1, P), np.float16),
        onesbf=np.ones((1, P), f32).astype(bf16),
    )
    x = np.asarray(x, f32)
    in_maps = []
    for b in range(B):
        m = dict(shared)
        m["xr"] = np.ascontiguousarray(x[b])
        m["xt"] = np.ascontiguousarray(x[b].T)
        in_maps.append(m)
    return in_maps


def kernel(**inputs):
    if "nc" not in _CACHE:
        _CACHE["nc"] = build()
    nc = _CACHE["nc"]
    in_maps = make_in_maps(**inputs)
    res = bass_utils.run_bass_kernel_spmd(nc, in_maps, core_ids=list(range(B)))
    out = np.stack([r["out"] for r in res.results])
    attn = np.stack([r["attn"] for r in res.results])
    return out, attn
